# revision 35
# baseline (speedup 1.0000x reference)
"""Trainium2 Bass kernel for nn_MultiHeadAttn (B=2, S=2048, D=1024, H=16,
ADIM=64, rel-pos bias vocab 33).

Sharding: batch x head-group over 8 cores. Core c handles batch b=c//4 and
heads [4*(c%4), 4*(c%4)+4). Each core computes q/k/v projections for its 256
model dims, attention for its 4 heads, and a partial output projection; the
host sums the 4 partials per batch.

Attention pipeline (per head-pair mt, t-half th):
  - scoresT[s,t] = (q_t/8).k_s with k VARIANTS folding the far-field rel-pos
    bias (kLo = k + pemb[32] for s-t >= 256, kHi = k + pemb[0] for t-s >=
    256); the 3 diagonal-crossing 128-wide t-subtiles get their bias
    multiplicatively after exp via a host-precomputed band.
  - The two heads of a pair run their K=64 score matmuls CONCURRENTLY in the
    PE array (even head on rows 0-63, odd on 64-127 via tile_position
    auto-derived from base partitions), into separate psum tiles.
  - exp on ScalarE (the kernel's bottleneck engine: ~128 x [128,1024] tiles).
  - AV uses v as the STATIONARY operand ([s=128, 65] with a ones column) and
    streams expT as rhs at N=512, accumulating ctxT[d(+den), t] in psum
    across all 16 s-blocks. This moves exp(S x S) through the PE at 1
    col/cycle with only 4 matmuls per (head, st, th) and yields ctx already
    TRANSPOSED for the output projection (no PE transposes).
  - softmax denominator = ctxT row 64; reciprocal on DVE (partition 64),
    broadcast to partitions 0-63 with a K=1 outer-product matmul
    (lhsT=ones[1,64] at base partition 64), one aligned DVE multiply
    normalizes into SBUF bf16. The odd head's 64 rows are then shifted to
    partitions 64-127 by a small SBUF->SBUF DMA so the out-projection can
    contract K=128 over the pair.

All inputs are pre-swizzled on the host into the exact SBUF layouts so every
load is one large 2D DMA.
"""
import numpy as np
import ml_dtypes

import concourse.bacc as bacc
import concourse.mybir as mybir
import concourse.tile as tile
from concourse.bass_utils import run_bass_kernel_spmd

B, S, D = 2, 2048, 1024
H, ADIM, K_REL, NJ = 16, 64, 16, 33
HPC = 4            # heads per core
DHC = HPC * ADIM   # 256 model dims per core
P = 128
NST = S // P       # 16 s-tiles
NKC = D // P       # 8 contraction chunks for projections
BF16 = mybir.dt.bfloat16
FP32 = mybir.dt.float32

_COMPILED = None


def build_nc():
    nc = bacc.Bacc(None, target_bir_lowering=False)
    with tile.TileContext(nc) as tc:
        # DRAM I/O (shapes already in SBUF layout, see _host_inputs)
        x_d = {nm: nc.dram_tensor(f"x{nm}", [P, NKC * S], BF16,
                                  kind="ExternalInput") for nm in "qkv"}
        w_d = {nm: nc.dram_tensor(f"w{nm}", [P, NKC * DHC], BF16,
                                  kind="ExternalInput") for nm in "qkv"}
        wo_d = nc.dram_tensor("wo", [P, 2 * D], BF16, kind="ExternalInput")
        pemb0_d = nc.dram_tensor("pemb0", [P, 1], FP32, kind="ExternalInput")
        pemb32_d = nc.dram_tensor("pemb32", [P, 1], FP32, kind="ExternalInput")
        band_d = nc.dram_tensor("band", [HPC, P, NST * 3 * P], BF16,
                                kind="ExternalInput")
        out_d = nc.dram_tensor("out", [S, D], BF16, kind="ExternalOutput")

        from contextlib import ExitStack
        with ExitStack() as stack:
            const = stack.enter_context(tc.tile_pool(name="const", bufs=1))
            pemb0_sb = const.tile([P, 1], FP32)
            pemb32_sb = const.tile([P, 1], FP32)
            ones_sb = const.tile([P, ADIM], BF16)
            nc.sync.dma_start(out=pemb0_sb[:], in_=pemb0_d[:])
            nc.sync.dma_start(out=pemb32_sb[:], in_=pemb32_d[:])
            nc.vector.memset(ones_sb[:], 1.0)

            persist = stack.enter_context(tc.tile_pool(name="persist", bufs=1))
            qT_sb = [persist.tile([P, S], BF16, name=f"qT{i}") for i in range(2)]
            kT_sb = [persist.tile([P, S], BF16, name=f"kT{i}") for i in range(2)]
            kLo_sb = [persist.tile([P, S], BF16, name=f"kLo{i}") for i in range(2)]
            kHi_sb = [persist.tile([P, S], BF16, name=f"kHi{i}") for i in range(2)]
            v_sb = [persist.tile([P, HPC * P], BF16, name=f"v{st}")
                    for st in range(NST)]
            ctxT_sb = [persist.tile([P, S], BF16, name=f"ctxT{i}")
                       for i in range(2)]
            wo_sb = persist.tile([P, 2 * D], BF16, name="wo")

            ostage = stack.enter_context(tc.tile_pool(name="ostage", bufs=3))

            def emit_out(tt, opool):
                """output projection + store for one 128-row t-block"""
                ps = opool.tile([P, 1024], FP32, name="scores")
                for nb in range(2):
                    for mt in range(2):
                        nc.tensor.matmul(
                            ps[:, nb * 512:nb * 512 + 512],
                            lhsT=ctxT_sb[mt][:, tt * P:tt * P + P],
                            rhs=wo_sb[:, mt * D + nb * 512:
                                      mt * D + nb * 512 + 512],
                            start=(mt == 0), stop=(mt == 1))
                st_t = ostage.tile([P, D], BF16, name="ost")
                nc.vector.tensor_copy(st_t[:], ps[:])
                nc.sync.dma_start(out=out_d[tt * P:tt * P + P, :], in_=st_t[:])

            # ---------------- P1: projections ----------------
            # x staging pools nest LIFO (xq innermost) so xq/xk free early
            # enough for the attention pools to reuse their SBUF while the
            # v projection still runs under the first attention block.
            with ExitStack() as p1:
                w_in = p1.enter_context(tc.tile_pool(name="w_in", bufs=1))
                ppsum = p1.enter_context(
                    tc.tile_pool(name="ppsum", bufs=4, space="PSUM"))
                w_sb = {}
                for nm in "qkv":
                    w_sb[nm] = w_in.tile([P, NKC * DHC], BF16, name=f"w{nm}")

                def proj_qk(nm, mt, x_t):
                    dst = qT_sb if nm == "q" else kT_sb
                    for nb in range(4):
                        ps = ppsum.tile([P, 512], FP32, name="proj")
                        for kc in range(NKC):
                            nc.tensor.matmul(
                                ps[:],
                                lhsT=w_sb[nm][:, kc * DHC + mt * P:
                                              kc * DHC + mt * P + P],
                                rhs=x_t[:, kc * S + nb * 512:
                                        kc * S + nb * 512 + 512],
                                start=(kc == 0), stop=(kc == NKC - 1))
                        # q copies ride the (P1-idle) scalar engine, k stays
                        # on DVE so neither engine gates P1
                        if nm == "q":
                            nc.scalar.activation(
                                dst[mt][:, nb * 512:nb * 512 + 512],
                                ps[:], mybir.ActivationFunctionType.Copy,
                                scale=0.125)
                        else:
                            nc.vector.tensor_copy(
                                dst[mt][:, nb * 512:nb * 512 + 512], ps[:])

                def kvariants(mt):
                    nc.vector.tensor_scalar_add(
                        kHi_sb[mt][:], kT_sb[mt][:], pemb0_sb[:])
                    nc.vector.tensor_scalar_add(
                        kLo_sb[mt][:], kT_sb[mt][:], pemb32_sb[:])

                with tc.tile_pool(name="xin", bufs=1) as xin:
                    x_sb = {nm: xin.tile([P, NKC * S], BF16, name=f"x{nm}")
                            for nm in "qkv"}
                    # DMAs in consumption order
                    nchunk = {"q": 4, "k": 2, "v": 2}
                    for nm in "qkv":
                        nc.sync.dma_start(out=w_sb[nm][:], in_=w_d[nm][:])
                        w = NKC * S // nchunk[nm]
                        for ch in range(nchunk[nm]):
                            nc.sync.dma_start(
                                out=x_sb[nm][:, ch * w:(ch + 1) * w],
                                in_=x_d[nm][:, ch * w:(ch + 1) * w])
                    nc.sync.dma_start(out=wo_sb[:], in_=wo_d[:])
                    proj_qk("q", 0, x_sb["q"])
                    proj_qk("k", 0, x_sb["k"])
                    kvariants(0)
                    for st in range(NST):
                        ps = ppsum.tile([P, DHC], FP32, name="projv")
                        for kc in range(NKC):
                            nc.tensor.matmul(
                                ps[:],
                                lhsT=x_sb["v"][:, kc * S + st * P:
                                               kc * S + st * P + P],
                                rhs=w_sb["v"][:, kc * DHC:(kc + 1) * DHC],
                                start=(kc == 0), stop=(kc == NKC - 1))
                        nc.vector.memset(v_sb[st][:], 1.0)
                        for h in range(HPC):
                            nc.vector.tensor_copy(
                                v_sb[st][:, P * h:P * h + ADIM],
                                ps[:, ADIM * h:ADIM * h + ADIM])
                    proj_qk("q", 1, x_sb["q"])
                    proj_qk("k", 1, x_sb["k"])
                    kvariants(1)

            # ---------------- P3: attention ----------------
            with ExitStack() as p3:
                spsum = p3.enter_context(
                    tc.tile_pool(name="spsum", bufs=2, space="PSUM"))
                cpsum = p3.enter_context(
                    tc.tile_pool(name="cpsum", bufs=2, space="PSUM"))
                epool = p3.enter_context(tc.tile_pool(name="expT", bufs=10))
                rpool = p3.enter_context(tc.tile_pool(name="recip", bufs=2))
                bpool = p3.enter_context(tc.tile_pool(name="band", bufs=3))

                band_sb = []
                for h in range(HPC):
                    bt = bpool.tile([P, NST * 3 * P], BF16, name="band")
                    nc.sync.dma_start(out=bt[:], in_=band_d[h])
                    band_sb.append(bt)

                ksrc = (kT_sb, kLo_sb, kHi_sb)

                def emit_sc(mt, th, st):
                    """scores + exp + band for one (st); returns the exp
                    tiles so the AV matmuls can be emitted later (lagged)."""
                    t0 = th * 8
                    s0 = st * P
                    exps = []
                    for hb in range(2):
                        hh = 2 * mt + hb
                        po = ADIM * hb
                        ps = spsum.tile([P, 1024], FP32, name="scores")
                        runs = []
                        for tt in range(t0, t0 + 8):
                            dd = st - tt
                            kv = 1 if dd >= 2 else (2 if dd <= -2 else 0)
                            if (runs and runs[-1][2] == kv
                                    and (tt - t0) % 4 != 0):
                                runs[-1][1] = tt + 1
                            else:
                                runs.append([tt, tt + 1, kv])
                        for ta, tb, kv in runs:
                            co = (ta - t0) * P
                            nc.tensor.matmul(
                                ps[:, co:co + (tb - ta) * P],
                                lhsT=ksrc[kv][mt][po:po + ADIM, s0:s0 + P],
                                rhs=qT_sb[mt][po:po + ADIM, ta * P:tb * P],
                                start=True, stop=True)
                        expT = epool.tile([P, 1024], BF16, name="expT")
                        nc.scalar.activation(
                            expT[:], ps[:], mybir.ActivationFunctionType.Exp)
                        # multiplicative rel-pos band on the <=3 diagonal-
                        # crossing blocks, coalesced into one DVE op
                        lo = max(st - 1, t0)
                        hi = min(st + 1, t0 + 7)
                        if lo <= hi:
                            bo = (st * 3 + lo - (st - 1)) * P
                            co = (lo - t0) * P
                            w = (hi - lo + 1) * P
                            nc.vector.tensor_mul(
                                expT[:, co:co + w], expT[:, co:co + w],
                                band_sb[hh][:, bo:bo + w])
                        exps.append(expT)
                    return exps

                def emit_av(mt, st, ctx_ps, exps):
                    for hb in range(2):
                        hh = 2 * mt + hb
                        for nb in range(2):
                            nc.tensor.matmul(
                                ctx_ps[hb][:, nb * 512:nb * 512 + 512],
                                lhsT=v_sb[st][:, P * hh:P * hh + P],
                                rhs=exps[hb][:, nb * 512:nb * 512 + 512],
                                start=(st == 0), stop=(st == NST - 1))

                # normalize: den replicated on psum rows 64-127 by the ones
                # half of the AV weights. 1/den via bitcast seed + 1 Newton
                # pass (1x-rate DVE ALU ops; InstReciprocal is ~6.5us/call
                # and the approx_fast custom op miscompiles on this runtime;
                # residual ~0.7% against a 2e-2 budget), then a K=1 broadcast
                # matmul into the dead den rows and one DVE multiply per
                # head. The pieces are emitted interleaved into the NEXT
                # block's first st-steps so the in-order engine queues never
                # head-of-line block on the chain.
                def norm_dve(ctx_ps):
                    recs = []
                    for hb in range(2):
                        sd = rpool.tile([P, 1024], FP32, name="sd")
                        tmp = rpool.tile([P, 1024], FP32, name="tmp")
                        rec = rpool.tile([P, 1024], BF16, name="rec")
                        nc.vector.tensor_scalar(
                            sd[64:128, :].bitcast(mybir.dt.int32),
                            ctx_ps[hb][64:128, :].bitcast(mybir.dt.int32),
                            -1, 0x7EF311C3,
                            mybir.AluOpType.mult, mybir.AluOpType.add)
                        nc.vector.scalar_tensor_tensor(
                            tmp[64:128, :], ctx_ps[hb][64:128, :], -1.0,
                            sd[64:128, :], mybir.AluOpType.mult,
                            mybir.AluOpType.mult)
                        nc.vector.scalar_tensor_tensor(
                            rec[64:128, :], tmp[64:128, :], 2.0,
                            sd[64:128, :], mybir.AluOpType.add,
                            mybir.AluOpType.mult)
                        recs.append(rec)
                    return recs

                def norm_bcast(ctx_ps, recs):
                    for hb in range(2):
                        for nb in range(2):
                            nc.tensor.matmul(
                                ctx_ps[hb][64:128, nb * 512:nb * 512 + 512],
                                lhsT=ones_sb[64:65, :],
                                rhs=recs[hb][64:65, nb * 512:nb * 512 + 512],
                                start=True, stop=True)

                def norm_mul(mt, th, ctx_ps):
                    for hb in range(2):
                        bc_sb = rpool.tile([P, 1024], BF16, name="bcs")
                        nc.vector.tensor_copy(bc_sb[64:128, :],
                                              ctx_ps[hb][64:128, :])
                        nc.vector.tensor_mul(
                            ctxT_sb[mt][64 * hb:64 * hb + 64,
                                        th * 1024:th * 1024 + 1024],
                            ctx_ps[hb][0:64, :], bc_sb[64:128, :])

                LAG = 3
                pending = None
                for mt in range(2):
                    for th in range(2):
                        last = (mt, th) == (1, 1)
                        ctx_ps = [cpsum.tile([P, 1024], FP32, name="ctx")
                                  for _ in range(2)]
                        avq = []
                        for st in range(NST):
                            avq.append((st, emit_sc(mt, th, st)))
                            if pending is not None:
                                if st == 0:
                                    recs = norm_dve(pending[2])
                                elif st == 1:
                                    norm_bcast(pending[2], recs)
                                elif st == 2:
                                    norm_mul(*pending)
                                    pending = None
                            if len(avq) > LAG:
                                s, e = avq.pop(0)
                                emit_av(mt, s, ctx_ps, e)
                        for s, e in avq:
                            emit_av(mt, s, ctx_ps, e)
                        # th=0 output columns are final once the (1,0) chain
                        # lands -> out-proj for t-tiles 0-7 rides the last
                        # block's drain window on borrowed spsum tiles,
                        # overlapping the final normalize chain
                        if last:
                            for tt in range(8):
                                emit_out(tt, spsum)
                        pending = (mt, th, ctx_ps)
                recs2 = norm_dve(pending[2])
                norm_bcast(pending[2], recs2)
                norm_mul(*pending)

            # ---------------- P4: output projection (t-tiles 8-15) --------
            with ExitStack() as p4:
                opsum = p4.enter_context(
                    tc.tile_pool(name="opsum", bufs=2, space="PSUM"))
                for tt in range(8, NST):
                    emit_out(tt, opsum)
    nc.compile()
    return nc


def _bf16(x):
    return np.ascontiguousarray(np.asarray(x, np.float32)).astype(
        ml_dtypes.bfloat16)


def _swiz(xT):
    """[D, S]-like -> SBUF layout [128, (D/128)*S] (chunk kc at cols kc*S)."""
    d0, s0 = xT.shape
    return np.ascontiguousarray(
        xT.reshape(d0 // P, P, s0).transpose(1, 0, 2).reshape(P, -1))


def _host_inputs(iQ, iK, iV, Wq, Wk, Wv, Wo, rel_pemb):
    iQ, iK, iV = (np.asarray(a, np.float32) for a in (iQ, iK, iV))
    Wq, Wk, Wv, Wo = (np.asarray(a, np.float32) for a in (Wq, Wk, Wv, Wo))
    rel_pemb = np.asarray(rel_pemb, np.float32)
    pembT = rel_pemb.T
    pemb0 = np.tile(rel_pemb[0], 2).reshape(P, 1).astype(np.float32)
    pemb32 = np.tile(rel_pemb[32], 2).reshape(P, 1).astype(np.float32)

    sl = np.arange(P)[:, None]
    tl = np.arange(P)[None, :]
    idx_d = {d: np.clip(d + sl - tl + K_REL, 0, NJ - 1) for d in (128, 0, -128)}
    slot_d = (128, 0, -128)

    in_maps = []
    for c in range(8):
        b, g = c // 4, c % 4
        cols = slice(DHC * g, DHC * g + DHC)
        Qg = (iQ[b] @ Wq[:, cols]) * 0.125
        band = np.zeros((HPC, NST, 3, P, P), np.float32)
        for h in range(HPC):
            ph = Qg[:, ADIM * h:ADIM * h + ADIM] @ pembT
            for st in range(NST):
                for slot, d in enumerate(slot_d):
                    tt = st - 1 + slot
                    if not 0 <= tt < NST:
                        continue
                    pb = ph[tt * P:tt * P + P]
                    band[h, st, slot] = pb[tl, idx_d[d]]
        band = np.exp(band)
        # -> [HPC, 128(sl), NST*3*128(tl-groups)]
        band = np.ascontiguousarray(band.transpose(0, 3, 1, 2, 4)
                                    .reshape(HPC, P, NST * 3 * P))
        in_maps.append({
            "xq": _bf16(_swiz(iQ[b].T)), "xk": _bf16(_swiz(iK[b].T)),
            "xv": _bf16(_swiz(iV[b].T)),
            "wq": _bf16(_swiz(Wq[:, cols])), "wk": _bf16(_swiz(Wk[:, cols])),
            "wv": _bf16(_swiz(Wv[:, cols])), "wo": _bf16(_swiz(Wo[cols, :])),
            "pemb0": pemb0, "pemb32": pemb32, "band": _bf16(band),
        })
    return in_maps


def kernel(iQ, iK, iV, Wq, Wk, Wv, Wo, rel_pemb, _trace=False):
    global _COMPILED
    if _COMPILED is None:
        _COMPILED = build_nc()
    nc = _COMPILED
    in_maps = _host_inputs(iQ, iK, iV, Wq, Wk, Wv, Wo, rel_pemb)
    res = run_bass_kernel_spmd(nc, in_maps, list(range(8)), trace=_trace)
    parts = [res.results[c]["out"].astype(np.float32) for c in range(8)]
    out = np.stack([parts[0] + parts[1] + parts[2] + parts[3],
                    parts[4] + parts[5] + parts[6] + parts[7]])
    if _trace:
        return out, res
    return out


# revision 40
# speedup vs baseline: 1.0156x; 1.0156x over previous
"""Trainium2 Bass kernel for nn_MultiHeadAttn (B=2, S=2048, D=1024, H=16,
ADIM=64, rel-pos bias vocab 33).

Sharding: batch x head-group over 8 cores. Core c handles batch b=c//4 and
heads [4*(c%4), 4*(c%4)+4). Each core computes q/k/v projections for its 256
model dims, attention for its 4 heads, and a partial output projection; the
host sums the 4 partials per batch.

Attention pipeline (per head-pair mt, t-half th):
  - scoresT[s,t] = (q_t/8).k_s with k VARIANTS folding the far-field rel-pos
    bias (kLo = k + pemb[32] for s-t >= 256, kHi = k + pemb[0] for t-s >=
    256); the 3 diagonal-crossing 128-wide t-subtiles get their bias
    multiplicatively after exp via a host-precomputed band.
  - The two heads of a pair run their K=64 score matmuls CONCURRENTLY in the
    PE array (even head on rows 0-63, odd on 64-127 via tile_position
    auto-derived from base partitions), into separate psum tiles.
  - exp on ScalarE (the kernel's bottleneck engine: ~128 x [128,1024] tiles).
  - AV uses v as the STATIONARY operand ([s=128, 65] with a ones column) and
    streams expT as rhs at N=512, accumulating ctxT[d(+den), t] in psum
    across all 16 s-blocks. This moves exp(S x S) through the PE at 1
    col/cycle with only 4 matmuls per (head, st, th) and yields ctx already
    TRANSPOSED for the output projection (no PE transposes).
  - softmax denominator = ctxT row 64; reciprocal on DVE (partition 64),
    broadcast to partitions 0-63 with a K=1 outer-product matmul
    (lhsT=ones[1,64] at base partition 64), one aligned DVE multiply
    normalizes into SBUF bf16. The odd head's 64 rows are then shifted to
    partitions 64-127 by a small SBUF->SBUF DMA so the out-projection can
    contract K=128 over the pair.

All inputs are pre-swizzled on the host into the exact SBUF layouts so every
load is one large 2D DMA.
"""
import numpy as np
import ml_dtypes

import concourse.bacc as bacc
import concourse.mybir as mybir
import concourse.tile as tile
from concourse.bass_utils import run_bass_kernel_spmd

B, S, D = 2, 2048, 1024
H, ADIM, K_REL, NJ = 16, 64, 16, 33
HPC = 4            # heads per core
DHC = HPC * ADIM   # 256 model dims per core
P = 128
NST = S // P       # 16 s-tiles
NKC = D // P       # 8 contraction chunks for projections
BF16 = mybir.dt.bfloat16
FP32 = mybir.dt.float32

_COMPILED = None


def build_nc():
    nc = bacc.Bacc(None, target_bir_lowering=False)
    with tile.TileContext(nc) as tc:
        # DRAM I/O (shapes already in SBUF layout, see _host_inputs)
        x_d = {nm: nc.dram_tensor(f"x{nm}", [P, NKC * S], BF16,
                                  kind="ExternalInput") for nm in "qkv"}
        w_d = {nm: nc.dram_tensor(f"w{nm}", [P, NKC * DHC], BF16,
                                  kind="ExternalInput") for nm in "qkv"}
        wo_d = nc.dram_tensor("wo", [P, 2 * D], BF16, kind="ExternalInput")
        pemb0_d = nc.dram_tensor("pemb0", [P, 1], FP32, kind="ExternalInput")
        pemb32_d = nc.dram_tensor("pemb32", [P, 1], FP32, kind="ExternalInput")
        band_d = nc.dram_tensor("band", [HPC, P, NST * 3 * P], BF16,
                                kind="ExternalInput")
        out_d = nc.dram_tensor("out", [S, D], BF16, kind="ExternalOutput")

        from contextlib import ExitStack
        with ExitStack() as stack:
            const = stack.enter_context(tc.tile_pool(name="const", bufs=1))
            pemb0_sb = const.tile([P, 1], FP32)
            pemb32_sb = const.tile([P, 1], FP32)
            ones_sb = const.tile([P, ADIM], BF16)
            nc.sync.dma_start(out=pemb0_sb[:], in_=pemb0_d[:])
            nc.sync.dma_start(out=pemb32_sb[:], in_=pemb32_d[:])
            nc.vector.memset(ones_sb[:], 1.0)

            persist = stack.enter_context(tc.tile_pool(name="persist", bufs=1))
            qT_sb = [persist.tile([P, S], BF16, name=f"qT{i}") for i in range(2)]
            kT_sb = [persist.tile([P, S], BF16, name=f"kT{i}") for i in range(2)]
            kLo_sb = [persist.tile([P, S], BF16, name=f"kLo{i}") for i in range(2)]
            kHi_sb = [persist.tile([P, S], BF16, name=f"kHi{i}") for i in range(2)]
            v_sb = [persist.tile([P, HPC * P], BF16, name=f"v{st}")
                    for st in range(NST)]
            ctxT_sb = [persist.tile([P, S], BF16, name=f"ctxT{i}")
                       for i in range(2)]
            wo_sb = persist.tile([P, 2 * D], BF16, name="wo")

            ostage = stack.enter_context(tc.tile_pool(name="ostage", bufs=3))

            def emit_out(tt, opool):
                """output projection + store for one 128-row t-block; the
                psum->sbuf copies alternate ACT/DVE so the tail is not
                serialized on one engine"""
                ps = opool.tile([P, 1024], FP32, name="scores")
                for nb in range(2):
                    for mt in range(2):
                        nc.tensor.matmul(
                            ps[:, nb * 512:nb * 512 + 512],
                            lhsT=ctxT_sb[mt][:, tt * P:tt * P + P],
                            rhs=wo_sb[:, mt * D + nb * 512:
                                      mt * D + nb * 512 + 512],
                            start=(mt == 0), stop=(mt == 1))
                st_t = ostage.tile([P, D], BF16, name="ost")
                if tt % 2 == 0:
                    nc.scalar.activation(st_t[:], ps[:],
                                         mybir.ActivationFunctionType.Copy)
                else:
                    nc.vector.tensor_copy(st_t[:], ps[:])
                nc.sync.dma_start(out=out_d[tt * P:tt * P + P, :], in_=st_t[:])

            # ---------------- P1: projections ----------------
            # x staging pools nest LIFO (xq innermost) so xq/xk free early
            # enough for the attention pools to reuse their SBUF while the
            # v projection still runs under the first attention block.
            with ExitStack() as p1:
                w_in = p1.enter_context(tc.tile_pool(name="w_in", bufs=1))
                ppsum = p1.enter_context(
                    tc.tile_pool(name="ppsum", bufs=4, space="PSUM"))
                w_sb = {}
                for nm in "qkv":
                    w_sb[nm] = w_in.tile([P, NKC * DHC], BF16, name=f"w{nm}")

                def proj_qk(nm, mt, x_t):
                    dst = qT_sb if nm == "q" else kT_sb
                    for nb in range(4):
                        ps = ppsum.tile([P, 512], FP32, name="proj")
                        for kc in range(NKC):
                            nc.tensor.matmul(
                                ps[:],
                                lhsT=w_sb[nm][:, kc * DHC + mt * P:
                                              kc * DHC + mt * P + P],
                                rhs=x_t[:, kc * S + nb * 512:
                                        kc * S + nb * 512 + 512],
                                start=(kc == 0), stop=(kc == NKC - 1))
                        # q copies ride the (P1-idle) scalar engine, k stays
                        # on DVE so neither engine gates P1
                        if nm == "q":
                            nc.scalar.activation(
                                dst[mt][:, nb * 512:nb * 512 + 512],
                                ps[:], mybir.ActivationFunctionType.Copy,
                                scale=0.125)
                        else:
                            nc.vector.tensor_copy(
                                dst[mt][:, nb * 512:nb * 512 + 512], ps[:])

                def kvariants(mt):
                    nc.vector.tensor_scalar_add(
                        kHi_sb[mt][:], kT_sb[mt][:], pemb0_sb[:])
                    nc.vector.tensor_scalar_add(
                        kLo_sb[mt][:], kT_sb[mt][:], pemb32_sb[:])

                with tc.tile_pool(name="xin", bufs=1) as xin:
                    x_sb = {nm: xin.tile([P, NKC * S], BF16, name=f"x{nm}")
                            for nm in "qkv"}
                    # DMAs in consumption order
                    nchunk = {"q": 4, "k": 2, "v": 2}
                    for nm in "qkv":
                        nc.sync.dma_start(out=w_sb[nm][:], in_=w_d[nm][:])
                        w = NKC * S // nchunk[nm]
                        for ch in range(nchunk[nm]):
                            nc.sync.dma_start(
                                out=x_sb[nm][:, ch * w:(ch + 1) * w],
                                in_=x_d[nm][:, ch * w:(ch + 1) * w])
                    nc.sync.dma_start(out=wo_sb[:], in_=wo_d[:])
                    proj_qk("q", 0, x_sb["q"])
                    proj_qk("k", 0, x_sb["k"])
                    kvariants(0)
                    for st in range(NST):
                        ps = ppsum.tile([P, DHC], FP32, name="projv")
                        for kc in range(NKC):
                            nc.tensor.matmul(
                                ps[:],
                                lhsT=x_sb["v"][:, kc * S + st * P:
                                               kc * S + st * P + P],
                                rhs=w_sb["v"][:, kc * DHC:(kc + 1) * DHC],
                                start=(kc == 0), stop=(kc == NKC - 1))
                        nc.vector.memset(v_sb[st][:], 1.0)
                        for h in range(HPC):
                            nc.vector.tensor_copy(
                                v_sb[st][:, P * h:P * h + ADIM],
                                ps[:, ADIM * h:ADIM * h + ADIM])
                    proj_qk("q", 1, x_sb["q"])
                    proj_qk("k", 1, x_sb["k"])
                    kvariants(1)

            # ---------------- P3: attention ----------------
            with ExitStack() as p3:
                spsum = p3.enter_context(
                    tc.tile_pool(name="spsum", bufs=2, space="PSUM"))
                cpsum = p3.enter_context(
                    tc.tile_pool(name="cpsum", bufs=2, space="PSUM"))
                epool = p3.enter_context(tc.tile_pool(name="expT", bufs=10))
                rpool = p3.enter_context(tc.tile_pool(name="recip", bufs=2))
                bpool = p3.enter_context(tc.tile_pool(name="band", bufs=3))

                band_sb = []
                for h in range(HPC):
                    bt = bpool.tile([P, NST * 3 * P], BF16, name="band")
                    nc.sync.dma_start(out=bt[:], in_=band_d[h])
                    band_sb.append(bt)

                ksrc = (kT_sb, kLo_sb, kHi_sb)

                def emit_sc(mt, th, st):
                    """scores + exp + band for one (st); returns the exp
                    tiles so the AV matmuls can be emitted later (lagged)."""
                    t0 = th * 8
                    s0 = st * P
                    exps = []
                    for hb in range(2):
                        hh = 2 * mt + hb
                        po = ADIM * hb
                        ps = spsum.tile([P, 1024], FP32, name="scores")
                        runs = []
                        for tt in range(t0, t0 + 8):
                            dd = st - tt
                            kv = 1 if dd >= 2 else (2 if dd <= -2 else 0)
                            if (runs and runs[-1][2] == kv
                                    and (tt - t0) % 4 != 0):
                                runs[-1][1] = tt + 1
                            else:
                                runs.append([tt, tt + 1, kv])
                        for ta, tb, kv in runs:
                            co = (ta - t0) * P
                            nc.tensor.matmul(
                                ps[:, co:co + (tb - ta) * P],
                                lhsT=ksrc[kv][mt][po:po + ADIM, s0:s0 + P],
                                rhs=qT_sb[mt][po:po + ADIM, ta * P:tb * P],
                                start=True, stop=True)
                        expT = epool.tile([P, 1024], BF16, name="expT")
                        nc.scalar.activation(
                            expT[:], ps[:], mybir.ActivationFunctionType.Exp)
                        # multiplicative rel-pos band on the <=3 diagonal-
                        # crossing blocks, coalesced into one DVE op
                        lo = max(st - 1, t0)
                        hi = min(st + 1, t0 + 7)
                        if lo <= hi:
                            bo = (st * 3 + lo - (st - 1)) * P
                            co = (lo - t0) * P
                            w = (hi - lo + 1) * P
                            nc.vector.tensor_mul(
                                expT[:, co:co + w], expT[:, co:co + w],
                                band_sb[hh][:, bo:bo + w])
                        exps.append(expT)
                    return exps

                def emit_av(mt, st, ctx_ps, exps):
                    for hb in range(2):
                        hh = 2 * mt + hb
                        for nb in range(2):
                            nc.tensor.matmul(
                                ctx_ps[hb][:, nb * 512:nb * 512 + 512],
                                lhsT=v_sb[st][:, P * hh:P * hh + P],
                                rhs=exps[hb][:, nb * 512:nb * 512 + 512],
                                start=(st == 0), stop=(st == NST - 1))

                # normalize: den replicated on psum rows 64-127 by the ones
                # half of the AV weights. 1/den via bitcast seed + 1 Newton
                # pass (1x-rate DVE ALU ops; InstReciprocal is ~6.5us/call
                # and the approx_fast custom op miscompiles on this runtime;
                # residual ~0.7% against a 2e-2 budget), then a K=1 broadcast
                # matmul into the dead den rows and one DVE multiply per
                # head. The pieces are emitted interleaved into the NEXT
                # block's first st-steps so the in-order engine queues never
                # head-of-line block on the chain.
                def norm_dve(ctx_ps):
                    recs = []
                    for hb in range(2):
                        sd = rpool.tile([P, 1024], FP32, name="sd")
                        tmp = rpool.tile([P, 1024], FP32, name="tmp")
                        rec = rpool.tile([P, 1024], BF16, name="rec")
                        nc.vector.tensor_scalar(
                            sd[64:128, :].bitcast(mybir.dt.int32),
                            ctx_ps[hb][64:128, :].bitcast(mybir.dt.int32),
                            -1, 0x7EF311C3,
                            mybir.AluOpType.mult, mybir.AluOpType.add)
                        nc.vector.scalar_tensor_tensor(
                            tmp[64:128, :], ctx_ps[hb][64:128, :], -1.0,
                            sd[64:128, :], mybir.AluOpType.mult,
                            mybir.AluOpType.mult)
                        nc.vector.scalar_tensor_tensor(
                            rec[64:128, :], tmp[64:128, :], 2.0,
                            sd[64:128, :], mybir.AluOpType.add,
                            mybir.AluOpType.mult)
                        recs.append(rec)
                    return recs

                def norm_bcast(ctx_ps, recs):
                    for hb in range(2):
                        for nb in range(2):
                            nc.tensor.matmul(
                                ctx_ps[hb][64:128, nb * 512:nb * 512 + 512],
                                lhsT=ones_sb[64:65, :],
                                rhs=recs[hb][64:65, nb * 512:nb * 512 + 512],
                                start=True, stop=True)

                def norm_mul(mt, th, ctx_ps):
                    for hb in range(2):
                        bc_sb = rpool.tile([P, 1024], BF16, name="bcs")
                        nc.vector.tensor_copy(bc_sb[64:128, :],
                                              ctx_ps[hb][64:128, :])
                        nc.vector.tensor_mul(
                            ctxT_sb[mt][64 * hb:64 * hb + 64,
                                        th * 1024:th * 1024 + 1024],
                            ctx_ps[hb][0:64, :], bc_sb[64:128, :])

                LAG = 3
                pending = None
                for mt in range(2):
                    for th in range(2):
                        last = (mt, th) == (1, 1)
                        ctx_ps = [cpsum.tile([P, 1024], FP32, name="ctx")
                                  for _ in range(2)]
                        avq = []
                        for st in range(NST):
                            avq.append((st, emit_sc(mt, th, st)))
                            if pending is not None:
                                if st == 0:
                                    recs = norm_dve(pending[2])
                                elif st == 1:
                                    norm_bcast(pending[2], recs)
                                elif st == 2:
                                    norm_mul(*pending)
                                    pending = None
                            if len(avq) > LAG:
                                s, e = avq.pop(0)
                                emit_av(mt, s, ctx_ps, e)
                        for s, e in avq:
                            emit_av(mt, s, ctx_ps, e)
                        # th=0 output columns are final once the (1,0) chain
                        # lands -> out-proj for t-tiles 0-7 rides the last
                        # block's drain window on borrowed spsum tiles,
                        # overlapping the final normalize chain
                        if last:
                            for tt in range(8):
                                emit_out(tt, spsum)
                        pending = (mt, th, ctx_ps)
                recs2 = norm_dve(pending[2])
                norm_bcast(pending[2], recs2)
                norm_mul(*pending)

            # ---------------- P4: output projection (t-tiles 8-15) --------
            with ExitStack() as p4:
                opsum = p4.enter_context(
                    tc.tile_pool(name="opsum", bufs=2, space="PSUM"))
                for tt in range(8, NST):
                    emit_out(tt, opsum)
    nc.compile()
    return nc


def _bf16(x):
    return np.ascontiguousarray(np.asarray(x, np.float32)).astype(
        ml_dtypes.bfloat16)


def _swiz(xT):
    """[D, S]-like -> SBUF layout [128, (D/128)*S] (chunk kc at cols kc*S)."""
    d0, s0 = xT.shape
    return np.ascontiguousarray(
        xT.reshape(d0 // P, P, s0).transpose(1, 0, 2).reshape(P, -1))


def _host_inputs(iQ, iK, iV, Wq, Wk, Wv, Wo, rel_pemb):
    iQ, iK, iV = (np.asarray(a, np.float32) for a in (iQ, iK, iV))
    Wq, Wk, Wv, Wo = (np.asarray(a, np.float32) for a in (Wq, Wk, Wv, Wo))
    rel_pemb = np.asarray(rel_pemb, np.float32)
    pembT = rel_pemb.T
    pemb0 = np.tile(rel_pemb[0], 2).reshape(P, 1).astype(np.float32)
    pemb32 = np.tile(rel_pemb[32], 2).reshape(P, 1).astype(np.float32)

    sl = np.arange(P)[:, None]
    tl = np.arange(P)[None, :]
    idx_d = {d: np.clip(d + sl - tl + K_REL, 0, NJ - 1) for d in (128, 0, -128)}
    slot_d = (128, 0, -128)

    in_maps = []
    for c in range(8):
        b, g = c // 4, c % 4
        cols = slice(DHC * g, DHC * g + DHC)
        Qg = (iQ[b] @ Wq[:, cols]) * 0.125
        band = np.zeros((HPC, NST, 3, P, P), np.float32)
        for h in range(HPC):
            ph = Qg[:, ADIM * h:ADIM * h + ADIM] @ pembT
            for st in range(NST):
                for slot, d in enumerate(slot_d):
                    tt = st - 1 + slot
                    if not 0 <= tt < NST:
                        continue
                    pb = ph[tt * P:tt * P + P]
                    band[h, st, slot] = pb[tl, idx_d[d]]
        band = np.exp(band)
        # -> [HPC, 128(sl), NST*3*128(tl-groups)]
        band = np.ascontiguousarray(band.transpose(0, 3, 1, 2, 4)
                                    .reshape(HPC, P, NST * 3 * P))
        in_maps.append({
            "xq": _bf16(_swiz(iQ[b].T)), "xk": _bf16(_swiz(iK[b].T)),
            "xv": _bf16(_swiz(iV[b].T)),
            "wq": _bf16(_swiz(Wq[:, cols])), "wk": _bf16(_swiz(Wk[:, cols])),
            "wv": _bf16(_swiz(Wv[:, cols])), "wo": _bf16(_swiz(Wo[cols, :])),
            "pemb0": pemb0, "pemb32": pemb32, "band": _bf16(band),
        })
    return in_maps


def kernel(iQ, iK, iV, Wq, Wk, Wv, Wo, rel_pemb, _trace=False):
    global _COMPILED
    if _COMPILED is None:
        _COMPILED = build_nc()
    nc = _COMPILED
    in_maps = _host_inputs(iQ, iK, iV, Wq, Wk, Wv, Wo, rel_pemb)
    res = run_bass_kernel_spmd(nc, in_maps, list(range(8)), trace=_trace)
    parts = [res.results[c]["out"].astype(np.float32) for c in range(8)]
    out = np.stack([parts[0] + parts[1] + parts[2] + parts[3],
                    parts[4] + parts[5] + parts[6] + parts[7]])
    if _trace:
        return out, res
    return out


# revision 50
# speedup vs baseline: 1.0427x; 1.0267x over previous
"""Trainium2 Bass kernel for nn_MultiHeadAttn (B=2, S=2048, D=1024, H=16,
ADIM=64, rel-pos bias vocab 33).

Sharding: batch x head-group over 8 cores. Core c handles batch b=c//4 and
heads [4*(c%4), 4*(c%4)+4). Each core computes q/k/v projections for its 256
model dims, attention for its 4 heads, and a partial output projection; the
host sums the 4 partials per batch.

Attention pipeline (per head-pair mt, t-half th):
  - scoresT[s,t] = (q_t/8).k_s with k VARIANTS folding the far-field rel-pos
    bias (kLo = k + pemb[32] for s-t >= 256, kHi = k + pemb[0] for t-s >=
    256); the 3 diagonal-crossing 128-wide t-subtiles get their bias
    multiplicatively after exp via a host-precomputed band.
  - The two heads of a pair run their K=64 score matmuls CONCURRENTLY in the
    PE array (even head on rows 0-63, odd on 64-127 via tile_position
    auto-derived from base partitions), into separate psum tiles.
  - exp on ScalarE (the kernel's bottleneck engine: ~128 x [128,1024] tiles).
  - AV uses v as the STATIONARY operand ([s=128, 65] with a ones column) and
    streams expT as rhs at N=512, accumulating ctxT[d(+den), t] in psum
    across all 16 s-blocks. This moves exp(S x S) through the PE at 1
    col/cycle with only 4 matmuls per (head, st, th) and yields ctx already
    TRANSPOSED for the output projection (no PE transposes).
  - softmax denominator = ctxT row 64; reciprocal on DVE (partition 64),
    broadcast to partitions 0-63 with a K=1 outer-product matmul
    (lhsT=ones[1,64] at base partition 64), one aligned DVE multiply
    normalizes into SBUF bf16. The odd head's 64 rows are then shifted to
    partitions 64-127 by a small SBUF->SBUF DMA so the out-projection can
    contract K=128 over the pair.

All inputs are pre-swizzled on the host into the exact SBUF layouts so every
load is one large 2D DMA.
"""
import numpy as np
import ml_dtypes

import concourse.bacc as bacc
import concourse.mybir as mybir
import concourse.tile as tile
from concourse.bass_utils import run_bass_kernel_spmd

B, S, D = 2, 2048, 1024
H, ADIM, K_REL, NJ = 16, 64, 16, 33
HPC = 4            # heads per core
DHC = HPC * ADIM   # 256 model dims per core
P = 128
NST = S // P       # 16 s-tiles
NKC = D // P       # 8 contraction chunks for projections
BF16 = mybir.dt.bfloat16
FP32 = mybir.dt.float32

_COMPILED = None


def build_nc():
    nc = bacc.Bacc(None, target_bir_lowering=False)
    with tile.TileContext(nc) as tc:
        # DRAM I/O (shapes already in SBUF layout, see _host_inputs)
        x_d = {nm: nc.dram_tensor(f"x{nm}", [P, NKC * S], BF16,
                                  kind="ExternalInput") for nm in "qkv"}
        w_d = {nm: nc.dram_tensor(f"w{nm}", [P, NKC * DHC], BF16,
                                  kind="ExternalInput") for nm in "qkv"}
        wo_d = nc.dram_tensor("wo", [P, 2 * D], BF16, kind="ExternalInput")
        pemb0_d = nc.dram_tensor("pemb0", [P, 1], FP32, kind="ExternalInput")
        pemb32_d = nc.dram_tensor("pemb32", [P, 1], FP32, kind="ExternalInput")
        band_d = nc.dram_tensor("band", [HPC, P, NST * 3 * P], BF16,
                                kind="ExternalInput")
        out_d = nc.dram_tensor("out", [S, D], BF16, kind="ExternalOutput")

        from contextlib import ExitStack
        with ExitStack() as stack:
            const = stack.enter_context(tc.tile_pool(name="const", bufs=1))
            pemb0_sb = const.tile([P, 1], FP32)
            pemb32_sb = const.tile([P, 1], FP32)
            ones_sb = const.tile([P, ADIM], BF16)
            nc.sync.dma_start(out=pemb0_sb[:], in_=pemb0_d[:])
            nc.sync.dma_start(out=pemb32_sb[:], in_=pemb32_d[:])
            nc.vector.memset(ones_sb[:], 1.0)

            persist = stack.enter_context(tc.tile_pool(name="persist", bufs=1))
            qT_sb = [persist.tile([P, S], BF16, name=f"qT{i}") for i in range(2)]
            kT_sb = [persist.tile([P, S], BF16, name=f"kT{i}") for i in range(2)]
            kLo_sb = [persist.tile([P, S], BF16, name=f"kLo{i}") for i in range(2)]
            kHi_sb = [persist.tile([P, S], BF16, name=f"kHi{i}") for i in range(2)]
            v_sb = [persist.tile([P, HPC * P], BF16, name=f"v{st}")
                    for st in range(NST)]
            ctxT_sb = [persist.tile([P, S], BF16, name=f"ctxT{i}")
                       for i in range(2)]
            wo_sb = persist.tile([P, 2 * D], BF16, name="wo")

            ostage = stack.enter_context(tc.tile_pool(name="ostage", bufs=3))

            def emit_out(tt, opool):
                """output projection + store for one 128-row t-block; the
                psum->sbuf copies alternate ACT/DVE so the tail is not
                serialized on one engine"""
                ps = opool.tile([P, 1024], FP32, name="scores")
                for nb in range(2):
                    for mt in range(2):
                        nc.tensor.matmul(
                            ps[:, nb * 512:nb * 512 + 512],
                            lhsT=ctxT_sb[mt][:, tt * P:tt * P + P],
                            rhs=wo_sb[:, mt * D + nb * 512:
                                      mt * D + nb * 512 + 512],
                            start=(mt == 0), stop=(mt == 1))
                st_t = ostage.tile([P, D], BF16, name="ost")
                if tt % 2 == 0:
                    nc.scalar.activation(st_t[:], ps[:],
                                         mybir.ActivationFunctionType.Copy)
                else:
                    nc.vector.tensor_copy(st_t[:], ps[:])
                nc.sync.dma_start(out=out_d[tt * P:tt * P + P, :], in_=st_t[:])

            # ---------------- P1: projections ----------------
            # x staging pools nest LIFO (xq innermost) so xq/xk free early
            # enough for the attention pools to reuse their SBUF while the
            # v projection still runs under the first attention block.
            with ExitStack() as p1:
                w_in = p1.enter_context(tc.tile_pool(name="w_in", bufs=1))
                ppsum = p1.enter_context(
                    tc.tile_pool(name="ppsum", bufs=4, space="PSUM"))
                w_sb = {}
                for nm in "qkv":
                    w_sb[nm] = w_in.tile([P, NKC * DHC], BF16, name=f"w{nm}")

                def proj_qk(nm, mt, x_t):
                    dst = qT_sb if nm == "q" else kT_sb
                    for nb in range(4):
                        ps = ppsum.tile([P, 512], FP32, name="proj")
                        for kc in range(NKC):
                            nc.tensor.matmul(
                                ps[:],
                                lhsT=w_sb[nm][:, kc * DHC + mt * P:
                                              kc * DHC + mt * P + P],
                                rhs=x_t[:, kc * S + nb * 512:
                                        kc * S + nb * 512 + 512],
                                start=(kc == 0), stop=(kc == NKC - 1))
                        # q copies ride the (P1-idle) scalar engine, k stays
                        # on DVE so neither engine gates P1
                        if nm == "q":
                            nc.scalar.activation(
                                dst[mt][:, nb * 512:nb * 512 + 512],
                                ps[:], mybir.ActivationFunctionType.Copy,
                                scale=0.125)
                        else:
                            nc.vector.tensor_copy(
                                dst[mt][:, nb * 512:nb * 512 + 512], ps[:])

                def kvariants(mt):
                    nc.vector.tensor_scalar_add(
                        kHi_sb[mt][:], kT_sb[mt][:], pemb0_sb[:])
                    nc.vector.tensor_scalar_add(
                        kLo_sb[mt][:], kT_sb[mt][:], pemb32_sb[:])

                with tc.tile_pool(name="xin", bufs=1) as xin:
                    x_sb = {nm: xin.tile([P, NKC * S], BF16, name=f"x{nm}")
                            for nm in "qkv"}
                    # DMAs in consumption order
                    nchunk = {"q": 4, "k": 2, "v": 2}
                    for nm in "qkv":
                        nc.sync.dma_start(out=w_sb[nm][:], in_=w_d[nm][:])
                        w = NKC * S // nchunk[nm]
                        for ch in range(nchunk[nm]):
                            nc.sync.dma_start(
                                out=x_sb[nm][:, ch * w:(ch + 1) * w],
                                in_=x_d[nm][:, ch * w:(ch + 1) * w])
                    nc.sync.dma_start(out=wo_sb[:], in_=wo_d[:])
                    proj_qk("q", 0, x_sb["q"])
                    proj_qk("k", 0, x_sb["k"])
                    kvariants(0)
                    for st in range(NST):
                        ps = ppsum.tile([P, DHC], FP32, name="projv")
                        for kc in range(NKC):
                            nc.tensor.matmul(
                                ps[:],
                                lhsT=x_sb["v"][:, kc * S + st * P:
                                               kc * S + st * P + P],
                                rhs=w_sb["v"][:, kc * DHC:(kc + 1) * DHC],
                                start=(kc == 0), stop=(kc == NKC - 1))
                        nc.vector.memset(v_sb[st][:], 1.0)
                        for h in range(HPC):
                            nc.vector.tensor_copy(
                                v_sb[st][:, P * h:P * h + ADIM],
                                ps[:, ADIM * h:ADIM * h + ADIM])
                    proj_qk("q", 1, x_sb["q"])
                    proj_qk("k", 1, x_sb["k"])
                    kvariants(1)

            # ---------------- P3: attention ----------------
            with ExitStack() as p3:
                spsum = p3.enter_context(
                    tc.tile_pool(name="spsum", bufs=2, space="PSUM"))
                cpsum = p3.enter_context(
                    tc.tile_pool(name="cpsum", bufs=2, space="PSUM"))
                epool = p3.enter_context(tc.tile_pool(name="expT", bufs=10))
                rpool = p3.enter_context(tc.tile_pool(name="recip", bufs=2))
                bpool = p3.enter_context(tc.tile_pool(name="band", bufs=3))

                band_sb = []
                for h in range(HPC):
                    bt = bpool.tile([P, NST * 3 * P], BF16, name="band")
                    nc.sync.dma_start(out=bt[:], in_=band_d[h])
                    band_sb.append(bt)

                ksrc = (kT_sb, kLo_sb, kHi_sb)

                def emit_sc(mt, th, st):
                    """scores + exp + band for one (st); returns the exp
                    tiles so the AV matmuls can be emitted later (lagged)."""
                    t0 = th * 8
                    s0 = st * P
                    exps = []
                    for hb in range(2):
                        hh = 2 * mt + hb
                        po = ADIM * hb
                        ps = spsum.tile([P, 1024], FP32, name="scores")
                        runs = []
                        for tt in range(t0, t0 + 8):
                            dd = st - tt
                            kv = 1 if dd >= 2 else (2 if dd <= -2 else 0)
                            if (runs and runs[-1][2] == kv
                                    and (tt - t0) % 4 != 0):
                                runs[-1][1] = tt + 1
                            else:
                                runs.append([tt, tt + 1, kv])
                        for ta, tb, kv in runs:
                            co = (ta - t0) * P
                            nc.tensor.matmul(
                                ps[:, co:co + (tb - ta) * P],
                                lhsT=ksrc[kv][mt][po:po + ADIM, s0:s0 + P],
                                rhs=qT_sb[mt][po:po + ADIM, ta * P:tb * P],
                                start=True, stop=True)
                        expT = epool.tile([P, 1024], BF16, name="expT")
                        nc.scalar.activation(
                            expT[:], ps[:], mybir.ActivationFunctionType.Exp)
                        # multiplicative rel-pos band on the <=3 diagonal-
                        # crossing blocks, coalesced into one DVE op
                        lo = max(st - 1, t0)
                        hi = min(st + 1, t0 + 7)
                        if lo <= hi:
                            bo = (st * 3 + lo - (st - 1)) * P
                            co = (lo - t0) * P
                            w = (hi - lo + 1) * P
                            nc.vector.tensor_mul(
                                expT[:, co:co + w], expT[:, co:co + w],
                                band_sb[hh][:, bo:bo + w])
                        exps.append(expT)
                    return exps

                def emit_av(mt, st, ctx_ps, exps):
                    for hb in range(2):
                        hh = 2 * mt + hb
                        for nb in range(2):
                            nc.tensor.matmul(
                                ctx_ps[hb][:, nb * 512:nb * 512 + 512],
                                lhsT=v_sb[st][:, P * hh:P * hh + P],
                                rhs=exps[hb][:, nb * 512:nb * 512 + 512],
                                start=(st == 0), stop=(st == NST - 1))

                # normalize: den replicated on psum rows 64-127 by the ones
                # half of the AV weights. 1/den via bitcast seed + 1 Newton
                # pass (1x-rate DVE ALU ops; InstReciprocal is ~6.5us/call
                # and the approx_fast custom op miscompiles on this runtime;
                # residual ~0.7% against a 2e-2 budget), then a K=1 broadcast
                # matmul into the dead den rows and one DVE multiply per
                # head. The pieces are emitted interleaved into the NEXT
                # block's first st-steps so the in-order engine queues never
                # head-of-line block on the chain.
                def norm_dve(ctx_ps):
                    recs = []
                    for hb in range(2):
                        sd = rpool.tile([P, 1024], FP32, name="sd")
                        tmp = rpool.tile([P, 1024], FP32, name="tmp")
                        rec = rpool.tile([P, 1024], BF16, name="rec")
                        nc.vector.tensor_scalar(
                            sd[64:128, :].bitcast(mybir.dt.int32),
                            ctx_ps[hb][64:128, :].bitcast(mybir.dt.int32),
                            -1, 0x7EF311C3,
                            mybir.AluOpType.mult, mybir.AluOpType.add)
                        nc.vector.scalar_tensor_tensor(
                            tmp[64:128, :], ctx_ps[hb][64:128, :], -1.0,
                            sd[64:128, :], mybir.AluOpType.mult,
                            mybir.AluOpType.mult)
                        nc.vector.scalar_tensor_tensor(
                            rec[64:128, :], tmp[64:128, :], 2.0,
                            sd[64:128, :], mybir.AluOpType.add,
                            mybir.AluOpType.mult)
                        recs.append(rec)
                    return recs

                def norm_bcast(ctx_ps, recs):
                    for hb in range(2):
                        for nb in range(2):
                            nc.tensor.matmul(
                                ctx_ps[hb][64:128, nb * 512:nb * 512 + 512],
                                lhsT=ones_sb[64:65, :],
                                rhs=recs[hb][64:65, nb * 512:nb * 512 + 512],
                                start=True, stop=True)

                def norm_mul(mt, th, ctx_ps):
                    for hb in range(2):
                        bc_sb = rpool.tile([P, 1024], BF16, name="bcs")
                        nc.vector.tensor_copy(bc_sb[64:128, :],
                                              ctx_ps[hb][64:128, :])
                        nc.vector.tensor_mul(
                            ctxT_sb[mt][64 * hb:64 * hb + 64,
                                        th * 1024:th * 1024 + 1024],
                            ctx_ps[hb][0:64, :], bc_sb[64:128, :])

                LAG = 3
                pending = None
                for mt in range(2):
                    for th in range(2):
                        last = (mt, th) == (1, 1)
                        ctx_ps = [cpsum.tile([P, 1024], FP32, name="ctx")
                                  for _ in range(2)]
                        avq = []
                        for st in range(NST):
                            avq.append((st, emit_sc(mt, th, st)))
                            if pending is not None:
                                if st == 0:
                                    recs = norm_dve(pending[2])
                                elif st == 1:
                                    norm_bcast(pending[2], recs)
                                elif st == 2:
                                    norm_mul(*pending)
                                    pending = None
                            if len(avq) > LAG:
                                s, e = avq.pop(0)
                                emit_av(mt, s, ctx_ps, e)
                        for s, e in avq:
                            emit_av(mt, s, ctx_ps, e)
                        # th=0 output columns are final once the (1,0) chain
                        # lands -> out-proj for t-tiles 0-7 rides the last
                        # block's drain window on borrowed spsum tiles,
                        # overlapping the final normalize chain
                        if last:
                            for tt in range(8):
                                emit_out(tt, spsum)
                        pending = (mt, th, ctx_ps)
                recs2 = norm_dve(pending[2])
                norm_bcast(pending[2], recs2)
                norm_mul(*pending)

            # ---------------- P4: output projection (t-tiles 8-15) --------
            with ExitStack() as p4:
                opsum = p4.enter_context(
                    tc.tile_pool(name="opsum", bufs=2, space="PSUM"))
                for tt in range(8, NST):
                    emit_out(tt, opsum)
    nc.compile()
    return nc


def _bf16(x):
    return np.ascontiguousarray(np.asarray(x, np.float32)).astype(
        ml_dtypes.bfloat16)


def _swiz(xT):
    """[D, S]-like -> SBUF layout [128, (D/128)*S] (chunk kc at cols kc*S)."""
    d0, s0 = xT.shape
    return np.ascontiguousarray(
        xT.reshape(d0 // P, P, s0).transpose(1, 0, 2).reshape(P, -1))


def _host_inputs(iQ, iK, iV, Wq, Wk, Wv, Wo, rel_pemb):
    iQ, iK, iV = (np.asarray(a, np.float32) for a in (iQ, iK, iV))
    Wq, Wk, Wv, Wo = (np.asarray(a, np.float32) for a in (Wq, Wk, Wv, Wo))
    rel_pemb = np.asarray(rel_pemb, np.float32)
    pembT = rel_pemb.T
    pemb0 = np.tile(rel_pemb[0], 2).reshape(P, 1).astype(np.float32)
    pemb32 = np.tile(rel_pemb[32], 2).reshape(P, 1).astype(np.float32)

    sl = np.arange(P)[:, None]
    tl = np.arange(P)[None, :]
    idx_d = {d: np.clip(d + sl - tl + K_REL, 0, NJ - 1) for d in (128, 0, -128)}
    slot_d = (128, 0, -128)

    in_maps = []
    for c in range(8):
        b, g = c // 4, c % 4
        cols = slice(DHC * g, DHC * g + DHC)
        Qg = (iQ[b] @ Wq[:, cols]) * 0.125
        band = np.zeros((HPC, NST, 3, P, P), np.float32)
        for h in range(HPC):
            ph = Qg[:, ADIM * h:ADIM * h + ADIM] @ pembT
            for st in range(NST):
                for slot, d in enumerate(slot_d):
                    tt = st - 1 + slot
                    if not 0 <= tt < NST:
                        continue
                    pb = ph[tt * P:tt * P + P]
                    band[h, st, slot] = pb[tl, idx_d[d]]
        band = np.exp(band)
        # -> [HPC, 128(sl), NST*3*128(tl-groups)]
        band = np.ascontiguousarray(band.transpose(0, 3, 1, 2, 4)
                                    .reshape(HPC, P, NST * 3 * P))
        in_maps.append({
            "xq": _bf16(_swiz(iQ[b].T)), "xk": _bf16(_swiz(iK[b].T)),
            "xv": _bf16(_swiz(iV[b].T)),
            "wq": _bf16(_swiz(Wq[:, cols])), "wk": _bf16(_swiz(Wk[:, cols])),
            "wv": _bf16(_swiz(Wv[:, cols])), "wo": _bf16(_swiz(Wo[cols, :])),
            "pemb0": pemb0, "pemb32": pemb32, "band": _bf16(band),
        })
    return in_maps


def kernel(iQ, iK, iV, Wq, Wk, Wv, Wo, rel_pemb, _trace=False):
    global _COMPILED
    if _COMPILED is None:
        _COMPILED = build_nc()
    nc = _COMPILED
    in_maps = _host_inputs(iQ, iK, iV, Wq, Wk, Wv, Wo, rel_pemb)
    res = run_bass_kernel_spmd(nc, in_maps, list(range(8)), trace=_trace)
    parts = [res.results[c]["out"].astype(np.float32) for c in range(8)]
    out = np.stack([parts[0] + parts[1] + parts[2] + parts[3],
                    parts[4] + parts[5] + parts[6] + parts[7]])
    if _trace:
        return out, res
    return out
